# revision 5
# baseline (speedup 1.0000x reference)
"""Trainium2 Bass kernel for a 3-layer edge-weighted GCN, 8 NeuronCores.

v4: instruction-count-optimized variant of the baseline scheme.
  - Same math as v1: agg = segsum(w * h[src]) per 128-dst window via
    one-hot scatter matmuls; dense W matmul after aggregation.
  - One-hots for a whole 4-window batch are built with TWO broadcast
    tensor_tensor ops (is_equal, mult) instead of one tensor_scalar per
    chunk (~45 DVE instructions per layer instead of ~586).
  - PSUM accumulates 4 windows per bank ([128, 512] f32); one batched
    PSUM->SBUF copy on the Activation engine per batch.
  - tabB gathered directly from hfull[HALF:] (no staging copy).
"""
import numpy as np
import ml_dtypes

import concourse.bass as bass
import concourse.bacc as bacc
import concourse.mybir as mybir
import concourse.tile as tile
from concourse.bass_utils import run_bass_kernel_spmd

N_NODES = 50000
N_EDGES = 500000
F = 128
HID = 128
OUT = 64
NCORES = 8

P = 128
NPC = N_NODES // NCORES            # 6250
NWIN = (NPC + P - 1) // P          # 49
NPAD = NWIN * P                    # 6272
NTOT = NCORES * NPAD               # 50176
HALF = (NCORES // 2) * NPAD        # 25088

GW = 8                             # windows per dma_gather batch
BQ = 4                             # windows per psum/one-hot batch

bf16 = mybir.dt.bfloat16
f32 = mybir.dt.float32
bfnp = ml_dtypes.bfloat16


def _wrap_idx(idx_flat):
    n = len(idx_flat)
    assert n % 128 == 0
    w = idx_flat.reshape(n // 16, 16).T.astype(np.int16)
    return np.ascontiguousarray(np.tile(w, (8, 1)))


def prep(x, src, dst, w1, w2, w3):
    """Host-side sharding/index prep. Returns (structure, in_maps)."""
    src = np.asarray(src).astype(np.int64)
    dst = np.asarray(dst).astype(np.int64)
    ws = [np.asarray(w, np.float32) for w in (w1, w2, w3)]

    src_pid = (src // NPC) * NPAD + (src % NPC)
    core = dst // NPC
    loc = dst % NPC
    win = loc // P
    doff = (loc % P).astype(np.float32)
    half = (src_pid >= HALF).astype(np.int64)

    cnt = np.zeros((NCORES, NWIN, 2), np.int64)
    np.add.at(cnt, (core, win, half), 1)
    nch = -(-cnt.max(axis=0) // P)
    for w in range(NWIN):
        if nch[w].sum() == 0:
            nch[w, 0] = 1
    ncha = int(nch[:, 0].sum())
    nchb = int(nch[:, 1].sum())
    ncht = ncha + nchb

    chunk_base = np.zeros((NWIN, 2), np.int64)
    run = 0
    for w in range(NWIN):
        chunk_base[w, 0] = run
        run += nch[w, 0]
        chunk_base[w, 1] = run
        run += nch[w, 1]
    assert run == ncht
    epad = ncht * P

    chunk_half = np.zeros(ncht, np.int64)
    for w in range(NWIN):
        chunk_half[chunk_base[w, 1]:chunk_base[w, 1] + nch[w, 1]] = 1
    a_cols = np.nonzero(chunk_half == 0)[0]
    b_cols = np.nonzero(chunk_half == 1)[0]

    gsrc = np.zeros((NCORES, epad), np.int64)
    dofa = np.zeros((NCORES, epad), np.float32)
    wfa = np.zeros((3, NCORES, epad), np.float32)
    for w in range(NWIN):
        s = chunk_base[w, 1] * P
        e = s + nch[w, 1] * P
        gsrc[:, s:e] = HALF

    order = np.lexsort((half, win, core))
    so_core = core[order]
    so_win = win[order]
    so_half = half[order]
    so_src = src_pid[order]
    so_doff = doff[order]
    so_w = [w[order] for w in ws]
    keys = (so_core * NWIN * 2 + so_win * 2 + so_half)
    startmask = np.ones(len(keys), bool)
    startmask[1:] = keys[1:] != keys[:-1]
    gstart = np.nonzero(startmask)[0]
    within = np.arange(len(keys)) - np.repeat(
        gstart, np.diff(np.append(gstart, len(keys))))
    pos = chunk_base[so_win, so_half] * P + within
    gsrc[so_core, pos] = so_src
    dofa[so_core, pos] = so_doff
    for i in range(3):
        wfa[i, so_core, pos] = so_w[i]

    def tr(a):
        return np.ascontiguousarray(a.reshape(ncht, P).T)

    xp = np.zeros((NTOT, F), bfnp)
    xf = np.asarray(x, np.float32)
    for c in range(NCORES):
        xp[c * NPAD:c * NPAD + NPC] = xf[c * NPC:(c + 1) * NPC].astype(bfnp)

    iota = np.broadcast_to(np.arange(P, dtype=np.float32), (P, P))

    in_maps = []
    for c in range(NCORES):
        idx_a = gsrc[c].reshape(ncht, P)[chunk_half == 0].ravel()
        idx_b = gsrc[c].reshape(ncht, P)[chunk_half == 1].ravel() - HALF
        if len(idx_b) == 0:
            idx_b = np.zeros(P, np.int64)
        in_maps.append({
            "xpA": np.ascontiguousarray(xp[:HALF]),
            "xpB": np.ascontiguousarray(xp[HALF:]),
            "idxA": _wrap_idx(idx_a),
            "idxB": _wrap_idx(idx_b),
            "doff": tr(dofa[c]).astype(bfnp),
            "wv1": tr(wfa[0, c]).astype(bfnp),
            "wv2": tr(wfa[1, c]).astype(bfnp),
            "wv3": tr(wfa[2, c]).astype(bfnp),
            "iota": np.ascontiguousarray(iota.astype(bfnp)),
        })

    struct = {
        "nch": nch, "chunk_base": chunk_base, "ncha": ncha, "nchb": nchb,
        "ncht": ncht, "a_cols": a_cols, "b_cols": b_cols,
    }
    return struct, in_maps


def build(struct, repeat=1):
    """Build the SPMD Bass program."""
    nch = struct["nch"]
    ncha, nchb, ncht = struct["ncha"], struct["nchb"], struct["ncht"]
    a_of = {g: i for i, g in enumerate(struct["a_cols"])}
    b_of = {g: i for i, g in enumerate(struct["b_cols"])}
    chunk_base = struct["chunk_base"]

    nc = bacc.Bacc("TRN2", target_bir_lowering=False, debug=False,
                   num_devices=NCORES)
    xpA = nc.dram_tensor("xpA", [HALF, F], bf16, kind="ExternalInput")
    xpB = nc.dram_tensor("xpB", [HALF, F], bf16, kind="ExternalInput")
    idxA = nc.dram_tensor("idxA", [P, ncha * 8], mybir.dt.int16,
                          kind="ExternalInput")
    idxB = nc.dram_tensor("idxB", [P, max(nchb, 1) * 8], mybir.dt.int16,
                          kind="ExternalInput")
    doff = nc.dram_tensor("doff", [P, ncht], bf16, kind="ExternalInput")
    wvs = [nc.dram_tensor(f"wv{i+1}", [P, ncht], bf16, kind="ExternalInput")
           for i in range(3)]
    iota = nc.dram_tensor("iota", [P, P], bf16, kind="ExternalInput")
    Wd = [nc.dram_tensor("W1", [F, HID], bf16, kind="ExternalInput"),
          nc.dram_tensor("W2", [HID, HID], bf16, kind="ExternalInput"),
          nc.dram_tensor("W3", [HID, OUT], bf16, kind="ExternalInput")]
    out = nc.dram_tensor("out", [NPAD, OUT], f32, kind="ExternalOutput")
    hpart = [nc.dram_tensor(f"hpart{l}", [NPAD, HID], bf16) for l in range(2)]
    hfull = [nc.dram_tensor(f"hfull{l}", [NTOT, HID], bf16,
                            addr_space="Shared") for l in range(2)]

    # bf16 copies of doff/iota for the one-hot compare (DVE 2x mode); doff
    # values are 0..127, exact in bf16.
    with tile.TileContext(nc) as tc:
        with (
            tc.tile_pool(name="const", bufs=1) as cst,
            tc.tile_pool(name="ma", bufs=3) as map_,
            tc.tile_pool(name="mb", bufs=3) as mbp,
            tc.tile_pool(name="oh", bufs=2) as ohp,
            tc.tile_pool(name="t1", bufs=2) as t1p,
            tc.tile_pool(name="agg", bufs=3) as aggp,
            tc.tile_pool(name="ho", bufs=4) as hop,
            tc.tile_pool(name="psa", bufs=3, space="PSUM") as psa,
            tc.tile_pool(name="pso", bufs=2, space="PSUM") as pso,
        ):
            idxA_sb = cst.tile([P, ncha * 8], mybir.dt.int16, tag="idxA")
            idxB_sb = cst.tile([P, max(nchb, 1) * 8], mybir.dt.int16,
                               tag="idxB")
            doff_sb = cst.tile([P, ncht], bf16, tag="doff")
            wv_sb = [cst.tile([P, ncht], bf16, tag=f"wv{i}",
                              name=f"wv{i}_sb") for i in range(3)]
            iota_sb = cst.tile([P, P], bf16, tag="iota")
            W_sb = [cst.tile([F, HID], bf16, tag="W1", name="W1_sb"),
                    cst.tile([HID, HID], bf16, tag="W2", name="W2_sb"),
                    cst.tile([HID, OUT], bf16, tag="W3", name="W3_sb")]
            nc.sync.dma_start(out=idxA_sb[:], in_=idxA[:, :])
            nc.sync.dma_start(out=idxB_sb[:], in_=idxB[:, :])
            nc.sync.dma_start(out=doff_sb[:], in_=doff[:, :])
            for i in range(3):
                nc.sync.dma_start(out=wv_sb[i][:], in_=wvs[i][:, :])
                nc.sync.dma_start(out=W_sb[i][:], in_=Wd[i][:, :])
            nc.sync.dma_start(out=iota_sb[:], in_=iota[:, :])

            wgroups = [list(range(g, min(g + GW, NWIN)))
                       for g in range(0, NWIN, GW)]

            for _ in range(repeat):
                for l in range(3):
                    tabA = (xpA[:, :] if l == 0
                            else hfull[l - 1][0:HALF, :])
                    tabB = (xpB[:, :] if l == 0
                            else hfull[l - 1][HALF:NTOT, :])
                    outf = HID if l < 2 else OUT
                    for grp in wgroups:
                        ga = [chunk_base[w, 0] + i
                              for w in grp for i in range(nch[w, 0])]
                        gb = [chunk_base[w, 1] + i
                              for w in grp for i in range(nch[w, 1])]
                        kA, kB = len(ga), len(gb)
                        a0 = a_of[ga[0]] if kA else 0
                        b0 = b_of[gb[0]] if kB else 0
                        mAt = mBt = None
                        if kA:
                            mAt = map_.tile([P, kA, F], bf16, tag="mA")
                            nc.gpsimd.dma_gather(
                                out_ap=mAt[:], in_ap=tabA,
                                idxs_ap=idxA_sb[:, a0 * 8:(a0 + kA) * 8],
                                num_idxs=kA * P, num_idxs_reg=kA * P,
                                elem_size=F, single_packet=False)
                        if kB:
                            mBt = mbp.tile([P, kB, F], bf16, tag="mB")
                            nc.gpsimd.dma_gather(
                                out_ap=mBt[:], in_ap=tabB,
                                idxs_ap=idxB_sb[:, b0 * 8:(b0 + kB) * 8],
                                num_idxs=kB * P, num_idxs_reg=kB * P,
                                elem_size=F, single_packet=False)
                        # process the group's windows in batches of BQ
                        for bs in range(0, len(grp), BQ):
                            batch = grp[bs:bs + BQ]
                            nb = len(batch)
                            g0 = int(chunk_base[batch[0], 0])
                            g1 = int(chunk_base[batch[-1], 1]
                                     + nch[batch[-1], 1])
                            kg = g1 - g0    # contiguous chunk range
                            # batched one-hot: oh[e, j, d] =
                            #   wv[e, g0+j] * (iota[d] == doff[e, g0+j])
                            t1 = t1p.tile([P, kg, P], bf16, tag="t1")
                            ohg = ohp.tile([P, kg, P], bf16, tag="oh")
                            iota_bc = iota_sb[:].unsqueeze(1).to_broadcast(
                                [P, kg, P])
                            doff_bc = doff_sb[:, g0:g1].unsqueeze(
                                2).to_broadcast([P, kg, P])
                            wv_bc = wv_sb[l][:, g0:g1].unsqueeze(
                                2).to_broadcast([P, kg, P])
                            nc.vector.tensor_tensor(
                                out=t1[:], in0=iota_bc, in1=doff_bc,
                                op=mybir.AluOpType.is_equal)
                            nc.vector.tensor_tensor(
                                out=ohg[:], in0=t1[:], in1=wv_bc,
                                op=mybir.AluOpType.mult)
                            pa4 = psa.tile([P, nb * P], f32, tag="pa")
                            for wi, w in enumerate(batch):
                                chunks = []
                                for i in range(nch[w, 0]):
                                    g = chunk_base[w, 0] + i
                                    chunks.append((mAt, a_of[g] - a0, g))
                                for i in range(nch[w, 1]):
                                    g = chunk_base[w, 1] + i
                                    chunks.append((mBt, b_of[g] - b0, g))
                                for j, (mt, lc, g) in enumerate(chunks):
                                    nc.tensor.matmul(
                                        pa4[:, wi * P:(wi + 1) * P],
                                        lhsT=mt[:, lc, :],
                                        rhs=ohg[:, g - g0, :],
                                        start=(j == 0),
                                        stop=(j == len(chunks) - 1))
                            aggT4 = aggp.tile([P, nb * P], bf16, tag="aggT")
                            nc.scalar.activation(
                                aggT4[:], pa4[:],
                                mybir.ActivationFunctionType.Copy)
                            po4 = pso.tile([P, nb, outf], f32, tag="po")
                            for wi, w in enumerate(batch):
                                nc.tensor.matmul(
                                    po4[:, wi, :],
                                    lhsT=aggT4[:, wi * P:(wi + 1) * P],
                                    rhs=W_sb[l][:, :],
                                    start=True, stop=True)
                            r0 = batch[0] * P
                            r1 = (batch[-1] + 1) * P
                            if l < 2:
                                ht4 = hop.tile([P, nb, HID], bf16, tag="ht")
                                nc.scalar.activation(
                                    ht4[:], po4[:],
                                    mybir.ActivationFunctionType.Relu)
                                oap = hpart[l][r0:r1, :].rearrange(
                                    "(wi p) f -> p wi f", wi=nb)
                                nc.sync.dma_start(out=oap, in_=ht4[:])
                            else:
                                ot4 = hop.tile([P, nb, OUT], f32, tag="ot")
                                nc.vector.tensor_copy(out=ot4[:], in_=po4[:])
                                oap = out[r0:r1, :].rearrange(
                                    "(wi p) f -> p wi f", wi=nb)
                                nc.sync.dma_start(out=oap, in_=ot4[:])
                    if l < 2:
                        nc.gpsimd.collective_compute(
                            "AllGather",
                            mybir.AluOpType.bypass,
                            replica_groups=[list(range(NCORES))],
                            ins=[hpart[l][:, :]],
                            outs=[hfull[l][:, :]],
                        )
    nc.compile()
    return nc


def _to_in_maps(in_maps, Wmats):
    W1, W2, W3 = Wmats
    for m in in_maps:
        m["W1"] = np.asarray(W1, np.float32).astype(bfnp)
        m["W2"] = np.asarray(W2, np.float32).astype(bfnp)
        m["W3"] = np.asarray(W3, np.float32).astype(bfnp)
    return in_maps


def kernel(x, src, dst, w1, w2, w3, W1, b1, W2, b2, W3, b3, _repeat=1,
           _prebuilt=None):
    if np.any(b1) or np.any(b2) or np.any(b3):
        raise NotImplementedError("nonzero biases not supported")
    struct, in_maps = prep(x, src, dst, w1, w2, w3)
    in_maps = _to_in_maps(in_maps, (W1, W2, W3))
    nc = _prebuilt or build(struct, repeat=_repeat)
    res = run_bass_kernel_spmd(nc, in_maps, list(range(NCORES)))
    outs = [res.results[c]["out"][:NPC] for c in range(NCORES)]
    return np.concatenate(outs, axis=0).astype(np.float32)


if __name__ == "__main__":
    pass


# revision 6
# speedup vs baseline: 1.2172x; 1.2172x over previous
"""Trainium2 Bass kernel for a 3-layer edge-weighted GCN, 8 NeuronCores.

v4: instruction-count-optimized variant of the baseline scheme.
  - Same math as v1: agg = segsum(w * h[src]) per 128-dst window via
    one-hot scatter matmuls; dense W matmul after aggregation.
  - One-hots for a whole 4-window batch are built with TWO broadcast
    tensor_tensor ops (is_equal, mult) instead of one tensor_scalar per
    chunk (~45 DVE instructions per layer instead of ~586).
  - PSUM accumulates 4 windows per bank ([128, 512] f32); one batched
    PSUM->SBUF copy on the Activation engine per batch.
  - tabB gathered directly from hfull[HALF:] (no staging copy).
"""
import numpy as np
import ml_dtypes

import concourse.bass as bass
import concourse.bacc as bacc
import concourse.mybir as mybir
import concourse.tile as tile
from concourse.bass_utils import run_bass_kernel_spmd

N_NODES = 50000
N_EDGES = 500000
F = 128
HID = 128
OUT = 64
NCORES = 8

P = 128
NPC = N_NODES // NCORES            # 6250
NWIN = (NPC + P - 1) // P          # 49
NPAD = NWIN * P                    # 6272
NTOT = NCORES * NPAD               # 50176
HALF = (NCORES // 2) * NPAD        # 25088

GW = 8                             # windows per dma_gather batch
BQ = 4                             # windows per psum/one-hot batch

bf16 = mybir.dt.bfloat16
f32 = mybir.dt.float32
bfnp = ml_dtypes.bfloat16


def _wrap_idx(idx_flat):
    n = len(idx_flat)
    assert n % 128 == 0
    w = idx_flat.reshape(n // 16, 16).T.astype(np.int16)
    return np.ascontiguousarray(np.tile(w, (8, 1)))


def prep(x, src, dst, w1, w2, w3):
    """Host-side sharding/index prep. Returns (structure, in_maps)."""
    src = np.asarray(src).astype(np.int64)
    dst = np.asarray(dst).astype(np.int64)
    ws = [np.asarray(w, np.float32) for w in (w1, w2, w3)]

    src_pid = (src // NPC) * NPAD + (src % NPC)
    core = dst // NPC
    loc = dst % NPC
    win = loc // P
    doff = (loc % P).astype(np.float32)
    half = (src_pid >= HALF).astype(np.int64)

    cnt = np.zeros((NCORES, NWIN, 2), np.int64)
    np.add.at(cnt, (core, win, half), 1)
    nch = -(-cnt.max(axis=0) // P)
    for w in range(NWIN):
        if nch[w].sum() == 0:
            nch[w, 0] = 1
    ncha = int(nch[:, 0].sum())
    nchb = int(nch[:, 1].sum())
    ncht = ncha + nchb

    chunk_base = np.zeros((NWIN, 2), np.int64)
    run = 0
    for w in range(NWIN):
        chunk_base[w, 0] = run
        run += nch[w, 0]
        chunk_base[w, 1] = run
        run += nch[w, 1]
    assert run == ncht
    epad = ncht * P

    chunk_half = np.zeros(ncht, np.int64)
    for w in range(NWIN):
        chunk_half[chunk_base[w, 1]:chunk_base[w, 1] + nch[w, 1]] = 1
    a_cols = np.nonzero(chunk_half == 0)[0]
    b_cols = np.nonzero(chunk_half == 1)[0]

    gsrc = np.zeros((NCORES, epad), np.int64)
    dofa = np.zeros((NCORES, epad), np.float32)
    wfa = np.zeros((3, NCORES, epad), np.float32)
    for w in range(NWIN):
        s = chunk_base[w, 1] * P
        e = s + nch[w, 1] * P
        gsrc[:, s:e] = HALF

    order = np.lexsort((half, win, core))
    so_core = core[order]
    so_win = win[order]
    so_half = half[order]
    so_src = src_pid[order]
    so_doff = doff[order]
    so_w = [w[order] for w in ws]
    keys = (so_core * NWIN * 2 + so_win * 2 + so_half)
    startmask = np.ones(len(keys), bool)
    startmask[1:] = keys[1:] != keys[:-1]
    gstart = np.nonzero(startmask)[0]
    within = np.arange(len(keys)) - np.repeat(
        gstart, np.diff(np.append(gstart, len(keys))))
    pos = chunk_base[so_win, so_half] * P + within
    gsrc[so_core, pos] = so_src
    dofa[so_core, pos] = so_doff
    for i in range(3):
        wfa[i, so_core, pos] = so_w[i]

    def tr(a):
        return np.ascontiguousarray(a.reshape(ncht, P).T)

    xp = np.zeros((NTOT, F), bfnp)
    xf = np.asarray(x, np.float32)
    for c in range(NCORES):
        xp[c * NPAD:c * NPAD + NPC] = xf[c * NPC:(c + 1) * NPC].astype(bfnp)

    in_maps = []
    for c in range(NCORES):
        idx_a = gsrc[c].reshape(ncht, P)[chunk_half == 0].ravel()
        idx_b = gsrc[c].reshape(ncht, P)[chunk_half == 1].ravel() - HALF
        if len(idx_b) == 0:
            idx_b = np.zeros(P, np.int64)
        # host-built one-hot weight tiles: oh_l[e, g, d] = wv_l[e,g] *
        # (doff[e,g] == d); streamed to the chunk matmuls from HBM.
        dof_i = tr(dofa[c]).astype(np.int64)[:, :, None]   # [P, ncht, 1]
        m = {
            "xpA": np.ascontiguousarray(xp[:HALF]),
            "xpB": np.ascontiguousarray(xp[HALF:]),
            "idxA": _wrap_idx(idx_a),
            "idxB": _wrap_idx(idx_b),
        }
        for i in range(3):
            ohv = np.zeros((P, ncht, P), bfnp)
            np.put_along_axis(
                ohv, dof_i,
                tr(wfa[i, c]).astype(bfnp)[:, :, None], axis=2)
            m[f"ohv{i+1}"] = ohv.reshape(P, ncht * P)
        in_maps.append(m)

    struct = {
        "nch": nch, "chunk_base": chunk_base, "ncha": ncha, "nchb": nchb,
        "ncht": ncht, "a_cols": a_cols, "b_cols": b_cols,
    }
    return struct, in_maps


def build(struct, repeat=1):
    """Build the SPMD Bass program."""
    nch = struct["nch"]
    ncha, nchb, ncht = struct["ncha"], struct["nchb"], struct["ncht"]
    a_of = {g: i for i, g in enumerate(struct["a_cols"])}
    b_of = {g: i for i, g in enumerate(struct["b_cols"])}
    chunk_base = struct["chunk_base"]

    nc = bacc.Bacc("TRN2", target_bir_lowering=False, debug=False,
                   num_devices=NCORES)
    xpA = nc.dram_tensor("xpA", [HALF, F], bf16, kind="ExternalInput")
    xpB = nc.dram_tensor("xpB", [HALF, F], bf16, kind="ExternalInput")
    idxA = nc.dram_tensor("idxA", [P, ncha * 8], mybir.dt.int16,
                          kind="ExternalInput")
    idxB = nc.dram_tensor("idxB", [P, max(nchb, 1) * 8], mybir.dt.int16,
                          kind="ExternalInput")
    ohvs = [nc.dram_tensor(f"ohv{i+1}", [P, ncht * P], bf16,
                           kind="ExternalInput") for i in range(3)]
    Wd = [nc.dram_tensor("W1", [F, HID], bf16, kind="ExternalInput"),
          nc.dram_tensor("W2", [HID, HID], bf16, kind="ExternalInput"),
          nc.dram_tensor("W3", [HID, OUT], bf16, kind="ExternalInput")]
    out = nc.dram_tensor("out", [NPAD, OUT], f32, kind="ExternalOutput")
    hpart = [nc.dram_tensor(f"hpart{l}", [NPAD, HID], bf16) for l in range(2)]
    hfull = [nc.dram_tensor(f"hfull{l}", [NTOT, HID], bf16,
                            addr_space="Shared") for l in range(2)]

    # bf16 copies of doff/iota for the one-hot compare (DVE 2x mode); doff
    # values are 0..127, exact in bf16.
    with tile.TileContext(nc) as tc:
        with (
            tc.tile_pool(name="const", bufs=1) as cst,
            tc.tile_pool(name="ma", bufs=3) as map_,
            tc.tile_pool(name="mb", bufs=3) as mbp,
            tc.tile_pool(name="oh", bufs=3) as ohp,
            tc.tile_pool(name="agg", bufs=3) as aggp,
            tc.tile_pool(name="ho", bufs=4) as hop,
            tc.tile_pool(name="psa", bufs=3, space="PSUM") as psa,
            tc.tile_pool(name="pso", bufs=2, space="PSUM") as pso,
        ):
            idxA_sb = cst.tile([P, ncha * 8], mybir.dt.int16, tag="idxA")
            idxB_sb = cst.tile([P, max(nchb, 1) * 8], mybir.dt.int16,
                               tag="idxB")
            W_sb = [cst.tile([F, HID], bf16, tag="W1", name="W1_sb"),
                    cst.tile([HID, HID], bf16, tag="W2", name="W2_sb"),
                    cst.tile([HID, OUT], bf16, tag="W3", name="W3_sb")]
            nc.sync.dma_start(out=idxA_sb[:], in_=idxA[:, :])
            nc.sync.dma_start(out=idxB_sb[:], in_=idxB[:, :])
            for i in range(3):
                nc.sync.dma_start(out=W_sb[i][:], in_=Wd[i][:, :])

            wgroups = [list(range(g, min(g + GW, NWIN)))
                       for g in range(0, NWIN, GW)]

            for _ in range(repeat):
                for l in range(3):
                    tabA = (xpA[:, :] if l == 0
                            else hfull[l - 1][0:HALF, :])
                    tabB = (xpB[:, :] if l == 0
                            else hfull[l - 1][HALF:NTOT, :])
                    outf = HID if l < 2 else OUT
                    for grp in wgroups:
                        ga = [chunk_base[w, 0] + i
                              for w in grp for i in range(nch[w, 0])]
                        gb = [chunk_base[w, 1] + i
                              for w in grp for i in range(nch[w, 1])]
                        kA, kB = len(ga), len(gb)
                        a0 = a_of[ga[0]] if kA else 0
                        b0 = b_of[gb[0]] if kB else 0
                        mAt = mBt = None
                        if kA:
                            mAt = map_.tile([P, kA, F], bf16, tag="mA")
                            nc.gpsimd.dma_gather(
                                out_ap=mAt[:], in_ap=tabA,
                                idxs_ap=idxA_sb[:, a0 * 8:(a0 + kA) * 8],
                                num_idxs=kA * P, num_idxs_reg=kA * P,
                                elem_size=F, single_packet=False)
                        if kB:
                            mBt = mbp.tile([P, kB, F], bf16, tag="mB")
                            nc.gpsimd.dma_gather(
                                out_ap=mBt[:], in_ap=tabB,
                                idxs_ap=idxB_sb[:, b0 * 8:(b0 + kB) * 8],
                                num_idxs=kB * P, num_idxs_reg=kB * P,
                                elem_size=F, single_packet=False)
                        # process the group's windows in batches of BQ
                        for bs in range(0, len(grp), BQ):
                            batch = grp[bs:bs + BQ]
                            nb = len(batch)
                            g0 = int(chunk_base[batch[0], 0])
                            g1 = int(chunk_base[batch[-1], 1]
                                     + nch[batch[-1], 1])
                            kg = g1 - g0    # contiguous chunk range
                            # host-precomputed one-hot weights from HBM
                            ohg = ohp.tile([P, kg, P], bf16, tag="oh")
                            nc.sync.dma_start(
                                out=ohg[:],
                                in_=ohvs[l][:, g0 * P:g1 * P])
                            pa4 = psa.tile([P, nb * P], f32, tag="pa")
                            for wi, w in enumerate(batch):
                                chunks = []
                                for i in range(nch[w, 0]):
                                    g = chunk_base[w, 0] + i
                                    chunks.append((mAt, a_of[g] - a0, g))
                                for i in range(nch[w, 1]):
                                    g = chunk_base[w, 1] + i
                                    chunks.append((mBt, b_of[g] - b0, g))
                                for j, (mt, lc, g) in enumerate(chunks):
                                    nc.tensor.matmul(
                                        pa4[:, wi * P:(wi + 1) * P],
                                        lhsT=mt[:, lc, :],
                                        rhs=ohg[:, g - g0, :],
                                        start=(j == 0),
                                        stop=(j == len(chunks) - 1))
                            aggT4 = aggp.tile([P, nb * P], bf16, tag="aggT")
                            nc.scalar.activation(
                                aggT4[:], pa4[:],
                                mybir.ActivationFunctionType.Copy)
                            po4 = pso.tile([P, nb, outf], f32, tag="po")
                            for wi, w in enumerate(batch):
                                nc.tensor.matmul(
                                    po4[:, wi, :],
                                    lhsT=aggT4[:, wi * P:(wi + 1) * P],
                                    rhs=W_sb[l][:, :],
                                    start=True, stop=True)
                            r0 = batch[0] * P
                            r1 = (batch[-1] + 1) * P
                            if l < 2:
                                ht4 = hop.tile([P, nb, HID], bf16, tag="ht")
                                nc.scalar.activation(
                                    ht4[:], po4[:],
                                    mybir.ActivationFunctionType.Relu)
                                oap = hpart[l][r0:r1, :].rearrange(
                                    "(wi p) f -> p wi f", wi=nb)
                                nc.sync.dma_start(out=oap, in_=ht4[:])
                            else:
                                ot4 = hop.tile([P, nb, OUT], f32, tag="ot")
                                nc.vector.tensor_copy(out=ot4[:], in_=po4[:])
                                oap = out[r0:r1, :].rearrange(
                                    "(wi p) f -> p wi f", wi=nb)
                                nc.sync.dma_start(out=oap, in_=ot4[:])
                    if l < 2:
                        nc.gpsimd.collective_compute(
                            "AllGather",
                            mybir.AluOpType.bypass,
                            replica_groups=[list(range(NCORES))],
                            ins=[hpart[l][:, :]],
                            outs=[hfull[l][:, :]],
                        )
    nc.compile()
    return nc


def _to_in_maps(in_maps, Wmats):
    W1, W2, W3 = Wmats
    for m in in_maps:
        m["W1"] = np.asarray(W1, np.float32).astype(bfnp)
        m["W2"] = np.asarray(W2, np.float32).astype(bfnp)
        m["W3"] = np.asarray(W3, np.float32).astype(bfnp)
    return in_maps


def kernel(x, src, dst, w1, w2, w3, W1, b1, W2, b2, W3, b3, _repeat=1,
           _prebuilt=None):
    if np.any(b1) or np.any(b2) or np.any(b3):
        raise NotImplementedError("nonzero biases not supported")
    struct, in_maps = prep(x, src, dst, w1, w2, w3)
    in_maps = _to_in_maps(in_maps, (W1, W2, W3))
    nc = _prebuilt or build(struct, repeat=_repeat)
    res = run_bass_kernel_spmd(nc, in_maps, list(range(NCORES)))
    outs = [res.results[c]["out"][:NPC] for c in range(NCORES)]
    return np.concatenate(outs, axis=0).astype(np.float32)


if __name__ == "__main__":
    pass
